# revision 2
# baseline (speedup 1.0000x reference)
"""Trainium2 Bass kernel for nn_Critic (additive-attention critic network).

Math (per sample, all folded on host):
  A    = UA @ x.T + biasA          UA = W_a@W_s           [f, tok]
  u1   = v_a . tanh(A)                                     [tok]
  a    = softmax(u1)  (constant-shift C1, exact softmax)
  G    = U1 @ x.T + a*(U2 @ x.T) + biasG                  [f, tok]
  u2   = v_c . tanh(G);  p2 = softmax(u2)
  y    = x.T @ p2   (h_i = W_s@y + b_s since sum(p2)=1)
  v    = W2 @ relu(W1 @ (W_s@y + b_s) + b1) + b2

Sharding: data-parallel over batch, 16 samples per core on 8 cores.
"""
import sys
import numpy as np

sys.path.insert(0, "/opt/trn_rl_repo")
import ml_dtypes  # noqa: E402
import concourse.bass as bass  # noqa: E402
import concourse.bacc as bacc  # noqa: E402
import concourse.mybir as mybir  # noqa: E402
import concourse.tile as tile  # noqa: E402
from concourse.bass_utils import run_bass_kernel_spmd  # noqa: E402
from contextlib import ExitStack  # noqa: E402

B, N, S, E = 128, 2048, 128, 128
NCORES, BLOC = 8, 16
bf16, f32 = mybir.dt.bfloat16, mybir.dt.float32
AF, ALU = mybir.ActivationFunctionType, mybir.AluOpType
GROUPS = [list(range(g, min(g + 3, BLOC))) for g in range(0, BLOC, 3)]

_cache = {}


def _build(trace=False):
    nc = bacc.Bacc("TRN2", target_bir_lowering=False, debug=False, num_devices=NCORES)
    x_d = nc.dram_tensor("x", [BLOC, N, S], bf16, kind="ExternalInput")
    uaT_d = nc.dram_tensor("uaT", [S, E], bf16, kind="ExternalInput")
    u1T_d = nc.dram_tensor("u1T", [S, E], bf16, kind="ExternalInput")
    u2T_d = nc.dram_tensor("u2T", [S, E], bf16, kind="ExternalInput")
    va_d = nc.dram_tensor("va", [E, 1], bf16, kind="ExternalInput")
    vc_d = nc.dram_tensor("vc", [E, 1], bf16, kind="ExternalInput")
    ones_d = nc.dram_tensor("ones", [128, 128], bf16, kind="ExternalInput")
    wsT_d = nc.dram_tensor("wsT", [S, E], f32, kind="ExternalInput")
    w1T_d = nc.dram_tensor("w1T", [E, E], f32, kind="ExternalInput")
    w2T_d = nc.dram_tensor("w2T", [E, 1], f32, kind="ExternalInput")
    # bias columns: 0 biasA, 1 biasG, 2 -C1, 3 -C2, 4 b_s, 5 b1, 6 b2
    bi_d = nc.dram_tensor("bi", [128, 8], f32, kind="ExternalInput")
    v_out = nc.dram_tensor("v", [1, BLOC], f32, kind="ExternalOutput")

    with tile.TileContext(nc) as tc, ExitStack() as ctx:
        cst = ctx.enter_context(tc.tile_pool(name="cst", bufs=1))
        xp = ctx.enter_context(tc.tile_pool(name="xp", bufs=6))
        ap = ctx.enter_context(tc.tile_pool(name="ap", bufs=3))
        ep = ctx.enter_context(tc.tile_pool(name="ep", bufs=2))
        sp = ctx.enter_context(tc.tile_pool(name="sp", bufs=8))
        pu = ctx.enter_context(tc.tile_pool(name="pu", bufs=1, space="PSUM"))
        pm = ctx.enter_context(tc.tile_pool(name="pm", bufs=2, space="PSUM"))

        uaT = cst.tile([S, E], bf16)
        nc.sync.dma_start(uaT[:], uaT_d.ap())
        u1T = cst.tile([S, E], bf16)
        nc.sync.dma_start(u1T[:], u1T_d.ap())
        u2T = cst.tile([S, E], bf16)
        nc.sync.dma_start(u2T[:], u2T_d.ap())
        va = cst.tile([E, 1], bf16)
        nc.sync.dma_start(va[:], va_d.ap())
        vc = cst.tile([E, 1], bf16)
        nc.sync.dma_start(vc[:], vc_d.ap())
        ones = cst.tile([128, 128], bf16)
        nc.sync.dma_start(ones[:], ones_d.ap())
        wsT = cst.tile([S, E], f32)
        nc.sync.dma_start(wsT[:], wsT_d.ap())
        w1T = cst.tile([E, E], f32)
        nc.sync.dma_start(w1T[:], w1T_d.ap())
        w2T = cst.tile([E, 1], f32)
        nc.sync.dma_start(w2T[:], w2T_d.ap())
        bi = cst.tile([128, 8], f32)
        nc.sync.dma_start(bi[:], bi_d.ap())
        ys = cst.tile([128, BLOC], f32)

        for grp in GROUPS:
            xts, tas = {}, {}
            for b, s in enumerate(grp):
                xt = xp.tile([128, N], bf16, tag="xt")
                nc.sync.dma_start_transpose(xt[:], x_d.ap()[s])
                xts[s] = xt

            # ---- branch 1: scores u1 for the group ----
            u_ps = pu.tile([128, N], f32, tag="u")
            for b, s in enumerate(grp):
                r = 32 * b
                ta = ap.tile([128, N], bf16, tag="tanh")
                for h in range(2):
                    mm = pm.tile([128, 1024], f32, tag="mm")
                    for q in range(2):
                        sl = slice(1024 * h + 512 * q, 1024 * h + 512 * (q + 1))
                        nc.tensor.matmul(mm[:, 512 * q:512 * (q + 1)], uaT[:],
                                         xts[s][:, sl], start=True, stop=True)
                    nc.scalar.activation(ta[:, 1024 * h:1024 * (h + 1)], mm[:],
                                         AF.Tanh, bias=bi[:, 0:1])
                for j in range(4):
                    nc.tensor.matmul(u_ps[r:r + 1, bass.ts(j, 512)], va[:],
                                     ta[:, bass.ts(j, 512)], start=True, stop=True,
                                     tile_position=(0, r))
            e1 = ep.tile([128, N], bf16, tag="e")
            z1 = sp.tile([128, 1], f32, tag="z")
            nc.scalar.activation(e1[:], u_ps[:], AF.Exp, bias=bi[:, 2:3], accum_out=z1[:])
            r1 = sp.tile([128, 1], f32, tag="z")
            nc.vector.reciprocal(r1[:], z1[:])
            an = ep.tile([128, N], bf16, tag="an")
            nc.vector.tensor_scalar_mul(an[:], e1[:], r1[:])

            # ---- branch 2: G = U1 x + a*(U2 x), scores u2 ----
            u2_ps = pu.tile([128, N], f32, tag="u")
            for b, s in enumerate(grp):
                r = 32 * b
                xa = ap.tile([128, N], bf16, tag="xa")
                for h in range(2):
                    ab = pm.tile([128, 1024], f32, tag="mm")
                    for q in range(2):
                        sl = slice(1024 * h + 512 * q, 1024 * h + 512 * (q + 1))
                        nc.tensor.matmul(ab[:, 512 * q:512 * (q + 1)], ones[r:r + 1, 0:128],
                                         an[r:r + 1, sl], start=True, stop=True,
                                         tile_position=(r, 0))
                    nc.vector.tensor_tensor(out=xa[:, 1024 * h:1024 * (h + 1)],
                                            in0=xts[s][:, 1024 * h:1024 * (h + 1)],
                                            in1=ab[:], op=ALU.mult)
                tg = ap.tile([128, N], bf16, tag="tanh")
                for h in range(2):
                    mm = pm.tile([128, 1024], f32, tag="mm")
                    for q in range(2):
                        sl = slice(1024 * h + 512 * q, 1024 * h + 512 * (q + 1))
                        nc.tensor.matmul(mm[:, 512 * q:512 * (q + 1)], u1T[:],
                                         xts[s][:, sl], start=True, stop=False)
                        nc.tensor.matmul(mm[:, 512 * q:512 * (q + 1)], u2T[:],
                                         xa[:, sl], start=False, stop=True)
                    nc.scalar.activation(tg[:, 1024 * h:1024 * (h + 1)], mm[:],
                                         AF.Tanh, bias=bi[:, 1:2])
                for j in range(4):
                    nc.tensor.matmul(u2_ps[r:r + 1, bass.ts(j, 512)], vc[:],
                                     tg[:, bass.ts(j, 512)], start=True, stop=True,
                                     tile_position=(0, r))
            e2 = ep.tile([128, N], bf16, tag="e")
            z2 = sp.tile([128, 1], f32, tag="z")
            nc.scalar.activation(e2[:], u2_ps[:], AF.Exp, bias=bi[:, 3:4], accum_out=z2[:])
            r2 = sp.tile([128, 1], f32, tag="z")
            nc.vector.reciprocal(r2[:], z2[:])
            pn = ep.tile([128, N], bf16, tag="an")
            nc.vector.tensor_scalar_mul(pn[:], e2[:], r2[:])

            # ---- y = x.T @ p2 per sample ----
            for b, s in enumerate(grp):
                r = 32 * b
                yh = []
                for h in range(2):
                    pb = pm.tile([128, 1024], f32, tag="mm")
                    for q in range(2):
                        sl = slice(1024 * h + 512 * q, 1024 * h + 512 * (q + 1))
                        nc.tensor.matmul(pb[:, 512 * q:512 * (q + 1)], ones[r:r + 1, 0:128],
                                         pn[r:r + 1, sl], start=True, stop=True,
                                         tile_position=(r, 0))
                    jk = ap.tile([128, 1024], bf16, tag="jk")
                    yp = sp.tile([128, 1], f32, tag="yp")
                    nc.vector.scalar_tensor_tensor(jk[:], xts[s][:, 1024 * h:1024 * (h + 1)],
                                                   1.0, pb[:], ALU.mult, ALU.mult,
                                                   accum_out=yp[:])
                    yh.append(yp)
                nc.vector.tensor_add(ys[:, s:s + 1], yh[0][:], yh[1][:])

        # ---- head: v = W2 relu(W1 (W_s y + b_s) + b1) + b2 ----
        hp = pm.tile([128, BLOC], f32, tag="mm")
        nc.tensor.matmul(hp[:], wsT[:], ys[:], start=True, stop=True)
        hs = sp.tile([128, BLOC], f32, tag="hd")
        nc.scalar.activation(hs[:], hp[:], AF.Identity, bias=bi[:, 4:5])
        op_ = pm.tile([128, BLOC], f32, tag="mm")
        nc.tensor.matmul(op_[:], w1T[:], hs[:], start=True, stop=True)
        os_ = sp.tile([128, BLOC], f32, tag="hd")
        nc.scalar.activation(os_[:], op_[:], AF.Relu, bias=bi[:, 5:6])
        vp = pm.tile([128, BLOC], f32, tag="mm")
        nc.tensor.matmul(vp[0:1, :], w2T[:], os_[:], start=True, stop=True)
        vs = sp.tile([1, BLOC], f32, tag="vs")
        nc.scalar.activation(vs[:], vp[0:1, :], AF.Identity, bias=bi[0:1, 6:7])
        nc.sync.dma_start(v_out.ap(), vs[:])

    nc.compile()
    return nc


def kernel(instance, W_s, b_s, W_a, b_a, v_a, W_c, b_c, v_c, W1, b1, W2, b2):
    if "nc" not in _cache:
        _cache["nc"] = _build()
    nc = _cache["nc"]

    f64 = np.float64
    Ws, Wa, Wc = W_s.astype(f64), W_a.astype(f64), W_c.astype(f64)
    UA = Wa @ Ws
    U1 = Wc[:, :E].astype(f64) @ Ws
    U2 = Wc[:, E:].astype(f64) @ Ws
    biasA = Wa @ b_s.astype(f64) + b_a.astype(f64)
    biasG = Wc[:, :E] @ b_s.astype(f64) + b_c.astype(f64)
    bias2 = Wc[:, E:] @ b_s.astype(f64)
    assert np.abs(bias2).max() < 1e-12, "nonzero W_c2@b_s not supported"
    C1 = max(0.0, float(np.abs(v_a.astype(f64)).sum()) - 60.0)
    C2 = max(0.0, float(np.abs(v_c.astype(f64)).sum()) - 60.0)

    bi = np.zeros((128, 8), np.float32)
    bi[:, 0] = biasA
    bi[:, 1] = biasG
    bi[:, 2] = -C1
    bi[:, 3] = -C2
    bi[:, 4] = b_s
    bi[:, 5] = b1
    bi[0, 6] = float(b2[0])

    bcast = {
        "uaT": np.ascontiguousarray(UA.T).astype(ml_dtypes.bfloat16),
        "u1T": np.ascontiguousarray(U1.T).astype(ml_dtypes.bfloat16),
        "u2T": np.ascontiguousarray(U2.T).astype(ml_dtypes.bfloat16),
        "va": v_a.reshape(E, 1).astype(ml_dtypes.bfloat16),
        "vc": v_c.reshape(E, 1).astype(ml_dtypes.bfloat16),
        "ones": np.ones((128, 128), ml_dtypes.bfloat16),
        "wsT": np.ascontiguousarray(Ws.T).astype(np.float32),
        "w1T": np.ascontiguousarray(W1.astype(f64).T).astype(np.float32),
        "w2T": np.ascontiguousarray(W2.astype(f64).T).astype(np.float32),
        "bi": bi,
    }
    xb = np.asarray(instance).astype(ml_dtypes.bfloat16)
    in_maps = [dict(bcast, x=np.ascontiguousarray(xb[c * BLOC:(c + 1) * BLOC]))
               for c in range(NCORES)]
    _cache["in_maps"] = in_maps
    res = run_bass_kernel_spmd(nc, in_maps, core_ids=list(range(NCORES)))
    _cache["last_results"] = res
    return np.concatenate([res.results[c]["v"][0] for c in range(NCORES)]).astype(np.float32)
